# revision 80
# baseline (speedup 1.0000x reference)
"""BatchHardTripletLoss on 8 Trainium2 NeuronCores.

Strategy (data parallel over rows; all reductions in squared-distance space;
sqrt is monotone so squared-space hardest-pos/neg selection is exact):

  Host: sort rows by label. Core c owns sorted rows [1024c, 1024c+1024).
  Columns (all 8192 candidates) are rotated per core so its own rows sit at
  fixed local columns [W/2, W/2+1024) -> all same-class pairs land in local
  columns [0, 128*(8+NMASK)) => one SPMD program for all 8 cores.

  Device work per core (engines balanced, PE streams bf16 matmuls):
   1) Row strip (local cols [0, 1024), most of the class band):
      TensorE psum[i,j] = -2 x_i.x_j; ScalarE raw-copies psum -> fp16 and
      ships to HBM (host applies +sq_j, class masks, strip reductions).
   2) Transposed region (local cols [1024, 8192), 56 j-tiles):
      TensorE psum[j,i] = -2 x_j.x_i (featsT block stationary, rows2
      moving), consumed in strict DVE/ACT alternation:
       - BC tiles: VectorE scalar_tensor_tensor (psum + sq_j) min run ->
         three interleaved fp16 running-min chains (no DMA traffic);
       - CS tiles: ScalarE raw copy -> fp16, batched 4 tiles per HBM DMA
         on the idle sync/gpsimd queues (host adds +sq_j).
      The NMASK band tiles overlapping [1024, 64+896+W) are forced to CS
      so the host can mask same-class entries.
  Host epilogue: min over tiles/partitions (numpy), class masks, + sq_i,
  clamp, sqrt, validity from label counts, margin + masked mean in fp32.
"""

import numpy as np
import ml_dtypes

N = 8192
D = 128
MARGIN = 0.3
NCORES = 8
ROWS_PER_CORE = N // NCORES          # 1024
RT_PER_CORE = ROWS_PER_CORE // 128   # 8 row-tiles
S = 1024                             # row-strip width
N_BC = 28                            # j-tiles on the DVE stt min-chains

_PROGRAM_CACHE = {}


def _mode_plan(TJT, n_mask):
    """Mode per j-tile: BC (DVE chain) / CS (ACT raw ship), alternating so
    both engines always have independent work; band-overlap tiles ship."""
    n_cs = TJT - N_BC
    modes = []
    nbc = ncs = 0
    for j in range(TJT):
        if j < n_mask:
            modes.append("CS")
            ncs += 1
        elif (j % 2 == 0 and nbc < N_BC) or ncs >= n_cs:
            modes.append("BC")
            nbc += 1
        else:
            modes.append("CS")
            ncs += 1
    return modes


def _build_program(W):
    import concourse.mybir as mybir
    from concourse import bacc
    from concourse.tile import TileContext

    F32 = mybir.dt.float32
    F16 = mybir.dt.float16
    BF16 = mybir.dt.bfloat16

    TJT = (N - S) // 128            # 56 transposed j-tiles
    WEND = 64 + 128 * 7 + W
    N_MASK = max(0, (WEND - S + 127) // 128)
    N_SHIP = TJT - N_BC
    NB_CHAINS = 3

    nc = bacc.Bacc("TRN2", target_bir_lowering=False, debug=False,
                   num_devices=NCORES)

    featsT_d = nc.dram_tensor("featsT", [D, N], BF16, kind="ExternalInput")
    rows2_d = nc.dram_tensor("rows2", [D, ROWS_PER_CORE], BF16,
                             kind="ExternalInput")
    sqT_d = nc.dram_tensor("sqT", [D, TJT], F32, kind="ExternalInput")

    strips_d = nc.dram_tensor("strips", [D, RT_PER_CORE * S], F16,
                              kind="ExternalOutput")
    tmins_d = nc.dram_tensor("tmins", [D, N_SHIP * ROWS_PER_CORE], F16,
                             kind="ExternalOutput")
    chains_d = nc.dram_tensor("chains", [D, NB_CHAINS * ROWS_PER_CORE], F16,
                              kind="ExternalOutput")

    modes = _mode_plan(TJT, N_MASK)

    # interleaved emission: two row-tiles up front (ACT has work while the
    # bulk featsT streams in), then one row-tile per ~8 j-tiles
    order = [("row", 0), ("row", 1)]
    jt = 0
    for lt in range(2, RT_PER_CORE):
        take = (TJT * (lt - 1)) // (RT_PER_CORE - 2) - \
               (TJT * (lt - 2)) // (RT_PER_CORE - 2)
        for _ in range(take):
            order.append(("jt", jt))
            jt += 1
        order.append(("row", lt))
    while jt < TJT:
        order.append(("jt", jt))
        jt += 1

    with TileContext(nc) as tc:
        with (
            tc.tile_pool(name="big", bufs=1) as big,
            tc.tile_pool(name="tps", bufs=4, space="PSUM") as tps_pool,
            tc.tile_pool(name="stp", bufs=2) as stp_pool,
            tc.tile_pool(name="shp", bufs=3) as shp_pool,
            tc.tile_pool(name="brn", bufs=2) as brn_pool,
            tc.tile_pool(name="small", bufs=1) as small,
        ):
            featsT = big.tile([D, N], BF16, tag="featsT")
            rows2 = big.tile([D, ROWS_PER_CORE], BF16, tag="rows2")
            sqT = small.tile([D, TJT], F32, tag="sqT")

            # ---- input DMAs: the critical prefix (rows2 + strip cols)
            # loads in parallel on two queues; bulk featsT rides BEHIND the
            # critical items on the same queues so it cannot starve them.
            nc.sync.dma_start(rows2[:, :], rows2_d[:, :])
            nc.gpsimd.dma_start(featsT[:, 0:S], featsT_d[:, 0:S])
            nc.scalar.dma_start(sqT[:, :], sqT_d[:, :])
            # small slice covering the first few j-tiles unblocks the DVE
            # chain path early
            nc.sync.dma_start(featsT[:, S:S + 1280], featsT_d[:, S:S + 1280])
            nc.sync.dma_start(featsT[:, S + 1280:3456],
                              featsT_d[:, S + 1280:3456])
            nc.gpsimd.dma_start(featsT[:, 3456:5824], featsT_d[:, 3456:5824])
            nc.scalar.dma_start(featsT[:, 5824:8192], featsT_d[:, 5824:8192])

            dma_flip = 0
            ship_k = 0
            shipbuf = None
            stripbuf = None
            brun = [[None, None] for _ in range(NB_CHAINS)]
            brun_n = [0] * NB_CHAINS
            bc_counts = [(N_BC + NB_CHAINS - 1 - ci) // NB_CHAINS
                         for ci in range(NB_CHAINS)]
            bc_k = 0
            chainbuf = small.tile([D, NB_CHAINS * ROWS_PER_CORE], F16,
                                  tag="chainbuf")

            def out_dma(dst, src):
                nonlocal dma_flip
                if dma_flip % 2 == 0:
                    nc.gpsimd.dma_start(dst, src)
                else:
                    nc.sync.dma_start(dst, src)
                dma_flip += 1

            for kind, idx in order:
                if kind == "row":
                    lt = idx
                    psA = tps_pool.tile([D, 1024], F32, tag="ps_t",
                                        name=f"psA{lt}")
                    lhsT = rows2[:, 128 * lt:128 * (lt + 1)]
                    nc.tensor.matmul(psA[:, 0:512], lhsT,
                                     featsT[:, 0:512],
                                     start=True, stop=True)
                    nc.tensor.matmul(psA[:, 512:1024], lhsT,
                                     featsT[:, 512:1024],
                                     start=True, stop=True)
                    if lt % 2 == 0:
                        stripbuf = stp_pool.tile([D, 2 * S], F16, tag="st",
                                                 name=f"st{lt}")
                    half = (lt % 2) * S
                    nc.scalar.activation(
                        stripbuf[:, half:half + S], psA[:, :],
                        mybir.ActivationFunctionType.Copy,
                        bias=0.0, scale=1.0)
                    if lt % 2 == 1:
                        out_dma(strips_d[:, (lt - 1) * S:(lt + 1) * S],
                                stripbuf[:, :])
                else:
                    jt = idx
                    j0 = S + 128 * jt
                    ps_t = tps_pool.tile([D, ROWS_PER_CORE], F32, tag="ps_t",
                                         name=f"ps_t{jt}")
                    lhsT = featsT[:, j0:j0 + 128]
                    nc.tensor.matmul(ps_t[:, 0:512], lhsT, rows2[:, 0:512],
                                     start=True, stop=True)
                    nc.tensor.matmul(ps_t[:, 512:1024], lhsT,
                                     rows2[:, 512:1024],
                                     start=True, stop=True)
                    mode = modes[jt]
                    if mode == "BC":
                        # DVE (psum + sq_j) min chain, fp16
                        ci = bc_k % NB_CHAINS
                        bc_k += 1
                        last = brun_n[ci] == bc_counts[ci] - 1
                        if last:
                            run = chainbuf[:, ci * ROWS_PER_CORE:
                                           (ci + 1) * ROWS_PER_CORE]
                        else:
                            run = brn_pool.tile(
                                [D, ROWS_PER_CORE], F16,
                                tag=f"brun{ci}", name=f"brun{jt}")[:, :]
                        if brun_n[ci] == 0:
                            nc.vector.tensor_scalar(
                                out=run, in0=ps_t[:, :],
                                scalar1=sqT[:, jt:jt + 1], scalar2=None,
                                op0=mybir.AluOpType.add)
                        else:
                            nc.vector.scalar_tensor_tensor(
                                out=run, in0=ps_t[:, :],
                                scalar=sqT[:, jt:jt + 1],
                                in1=brun[ci][(brun_n[ci] + 1) % 2],
                                op0=mybir.AluOpType.add,
                                op1=mybir.AluOpType.min)
                        brun[ci][brun_n[ci] % 2] = run
                        brun_n[ci] += 1
                    else:
                        # CS: ACT raw copy into a batched ship buffer
                        # (host adds +sq_j)
                        slot = ship_k % 4
                        if slot == 0:
                            shipbuf = shp_pool.tile(
                                [D, 4 * ROWS_PER_CORE], F16, tag="shipbuf",
                                name=f"shipbuf{ship_k}")
                        nc.scalar.activation(
                            shipbuf[:, slot * ROWS_PER_CORE:
                                    (slot + 1) * ROWS_PER_CORE],
                            ps_t[:, :],
                            mybir.ActivationFunctionType.Copy,
                            bias=0.0, scale=1.0)
                        ship_k += 1
                        if slot == 3:
                            b0 = (ship_k - 4) * ROWS_PER_CORE
                            out_dma(tmins_d[:, b0:b0 + 4 * ROWS_PER_CORE],
                                    shipbuf[:, :])

            # flush a partial ship batch
            rem = ship_k % 4
            if rem:
                b0 = (ship_k - rem) * ROWS_PER_CORE
                out_dma(tmins_d[:, b0:b0 + rem * ROWS_PER_CORE],
                        shipbuf[:, 0:rem * ROWS_PER_CORE])
            # chain finals were written straight into chainbuf
            nc.sync.dma_start(chains_d[:, :], chainbuf[:, :])

    nc.compile()
    return nc


def kernel(feats, labels):
    from concourse.bass_utils import run_bass_kernel_spmd

    feats = np.asarray(feats, dtype=np.float32)
    labels_np = np.asarray(labels).astype(np.int64)

    order = np.argsort(labels_np, kind="stable")
    feats_s = feats[order]
    labels_s = labels_np[order]

    counts = np.bincount(labels_s, minlength=max(int(labels_s.max()) + 1, 1))
    mc = int(counts.max())
    if mc <= 33:
        W = 192
    elif mc <= 65:
        W = 256
    elif mc <= 129:
        W = 384
    elif mc <= 193:
        W = 512
    else:
        raise ValueError(f"class of size {mc} exceeds supported band window")
    WEND = 64 + 128 * 7 + W
    NMASK = max(0, (WEND - S + 127) // 128)

    if W not in _PROGRAM_CACHE:
        _PROGRAM_CACHE[W] = _build_program(W)
    nc = _PROGRAM_CACHE[W]
    TJT = (N - S) // 128

    sq = np.einsum("nd,nd->n", feats_s.astype(np.float64),
                   feats_s.astype(np.float64)).astype(np.float32)

    in_maps = []
    locs = []
    for c in range(NCORES):
        rot = (ROWS_PER_CORE * c - W // 2) % N
        loc = (rot + np.arange(N)) % N          # local col -> global sorted row
        locs.append(loc)
        featsT_c = np.ascontiguousarray(
            feats_s[loc].T.astype(ml_dtypes.bfloat16))
        rows2_c = np.ascontiguousarray(
            (-2.0 * feats_s[ROWS_PER_CORE * c:ROWS_PER_CORE * (c + 1)])
            .T.astype(ml_dtypes.bfloat16))
        sq_loc = sq[loc]
        sqT_c = np.ascontiguousarray(
            sq_loc[S:S + TJT * 128].reshape(TJT, 128).T.astype(np.float32))
        in_maps.append({
            "featsT": featsT_c,
            "rows2": rows2_c,
            "sqT": sqT_c,
        })

    res = run_bass_kernel_spmd(nc, in_maps, core_ids=list(range(NCORES)))

    modes = _mode_plan(TJT, NMASK)
    cs_jts = np.array([j for j in range(TJT) if modes[j] == "CS"])

    neg_raw = np.empty(N, dtype=np.float32)
    pos_raw = np.empty(N, dtype=np.float32)
    for c in range(NCORES):
        base = ROWS_PER_CORE * c
        loc = locs[c]
        sq_loc = sq[loc]
        rows_lab = labels_s[base:base + ROWS_PER_CORE]
        # shipped tiles are RAW -2 x_j.x_i: add +sq_j here; chain tiles
        # already carry it from the device stt
        tm = res.results[c]["tmins"]             # [128, nship*1024] fp16
        ch = res.results[c]["chains"]            # [128, 3*1024] fp16
        nship = tm.shape[1] // ROWS_PER_CORE
        nchain = ch.shape[1] // ROWS_PER_CORE
        sq_ship = sq_loc[S + cs_jts[None, :] * 128 +
                         np.arange(128)[:, None]]          # [128, nship]
        tmf = tm.reshape(D, nship, ROWS_PER_CORE).astype(np.float32)
        tmf += sq_ship[:, :, None]
        # band ship-tiles: mask same-class entries (and collect their
        # hardest-pos candidates)
        pos_extra = np.full(ROWS_PER_CORE, -np.inf, dtype=np.float32)
        for k in range(NMASK):
            col_lab = labels_s[loc[S + 128 * k:S + 128 * (k + 1)]]
            msame = col_lab[:, None] == rows_lab[None, :]   # [128, 1024]
            tile = tmf[:, k, :]
            pos_extra = np.maximum(
                pos_extra, np.where(msame, tile, -np.inf).max(axis=0))
            tmf[:, k, :] = np.where(msame, np.inf, tile)
        tmin = np.minimum(tmf.min(axis=(0, 1)),
                          ch.reshape(D, nchain, ROWS_PER_CORE)
                            .astype(np.float32).min(axis=(0, 1)))
        # strip: raw -2 x_i . x_j for local cols [0, S)
        stp = res.results[c]["strips"].reshape(D, RT_PER_CORE, S)
        raw = stp.transpose(1, 0, 2).reshape(ROWS_PER_CORE, S) \
                 .astype(np.float32)
        vals = raw + sq_loc[None, 0:S]
        same = rows_lab[:, None] == labels_s[loc[0:S]][None, :]
        negs = np.where(same, np.float32(np.inf), vals).min(axis=1)
        poss = np.where(same, vals, np.float32(-np.inf)).max(axis=1)
        neg_raw[base:base + ROWS_PER_CORE] = np.minimum(negs, tmin)
        pos_raw[base:base + ROWS_PER_CORE] = np.maximum(poss, pos_extra)

    hn_sq = np.maximum(neg_raw + sq, 0.0).astype(np.float32)
    hp_sq = np.maximum(pos_raw + sq, 0.0).astype(np.float32)
    eps = np.float32(1e-12)
    hn = np.where(hn_sq > eps, np.sqrt(hn_sq), np.float32(0.0))
    hp = np.where(hp_sq > eps, np.sqrt(hp_sq), np.float32(0.0))

    cnt_per_row = counts[labels_s]
    valid = (cnt_per_row >= 2) & (cnt_per_row < N)
    diff = np.where(valid, hp - hn, np.float32(0.0))
    per_row = np.maximum(diff + np.float32(MARGIN), np.float32(0.0))
    per_row = np.where(valid, per_row, np.float32(0.0)).astype(np.float32)
    cnt = np.float32(valid.sum())
    if cnt > 0:
        loss = np.float32(per_row.sum(dtype=np.float32) /
                          max(cnt, np.float32(1.0)))
    else:
        loss = np.float32(0.0)
    return np.float32(loss)


# revision 81
# speedup vs baseline: 1.0214x; 1.0214x over previous
"""BatchHardTripletLoss on 8 Trainium2 NeuronCores.

Strategy (data parallel over rows; all reductions in squared-distance space;
sqrt is monotone so squared-space hardest-pos/neg selection is exact):

  Host: sort rows by label. Core c owns sorted rows [1024c, 1024c+1024).
  Columns (all 8192 candidates) are rotated per core so its own rows sit at
  fixed local columns [W/2, W/2+1024) -> all same-class pairs land in local
  columns [0, 128*(8+NMASK)) => one SPMD program for all 8 cores.

  Device work per core (engines balanced, PE streams bf16 matmuls):
   1) Row strip (local cols [0, 1024), most of the class band):
      TensorE psum[i,j] = -2 x_i.x_j; ScalarE raw-copies psum -> fp16 and
      ships to HBM (host applies +sq_j, class masks, strip reductions).
   2) Transposed region (local cols [1024, 8192), 56 j-tiles):
      TensorE psum[j,i] = -2 x_j.x_i (featsT block stationary, rows2
      moving), consumed in strict DVE/ACT alternation:
       - BC tiles: VectorE scalar_tensor_tensor (psum + sq_j) min run ->
         three interleaved fp16 running-min chains (no DMA traffic);
       - CS tiles: ScalarE raw copy -> fp16, batched 4 tiles per HBM DMA
         on the idle sync/gpsimd queues (host adds +sq_j).
      The NMASK band tiles overlapping [1024, 64+896+W) are forced to CS
      so the host can mask same-class entries.
  Host epilogue: min over tiles/partitions (numpy), class masks, + sq_i,
  clamp, sqrt, validity from label counts, margin + masked mean in fp32.
"""

import numpy as np
import ml_dtypes

N = 8192
D = 128
MARGIN = 0.3
NCORES = 8
ROWS_PER_CORE = N // NCORES          # 1024
RT_PER_CORE = ROWS_PER_CORE // 128   # 8 row-tiles
S = 1024                             # row-strip width
N_BC = 28                            # j-tiles on the DVE stt min-chains

_PROGRAM_CACHE = {}


def _mode_plan(TJT, n_mask):
    """Mode per j-tile: BC (DVE chain) / CS (ACT raw ship), alternating so
    both engines always have independent work; band-overlap tiles ship."""
    n_cs = TJT - N_BC
    modes = []
    nbc = ncs = 0
    for j in range(TJT):
        if j < n_mask:
            modes.append("CS")
            ncs += 1
        elif (j % 2 == 0 and nbc < N_BC) or ncs >= n_cs:
            modes.append("BC")
            nbc += 1
        else:
            modes.append("CS")
            ncs += 1
    return modes


def _build_program(W):
    import concourse.mybir as mybir
    from concourse import bacc
    from concourse.tile import TileContext

    F32 = mybir.dt.float32
    F16 = mybir.dt.float16
    BF16 = mybir.dt.bfloat16

    TJT = (N - S) // 128            # 56 transposed j-tiles
    WEND = 64 + 128 * 7 + W
    N_MASK = max(0, (WEND - S + 127) // 128)
    N_SHIP = TJT - N_BC
    NB_CHAINS = 3

    nc = bacc.Bacc("TRN2", target_bir_lowering=False, debug=False,
                   num_devices=NCORES)

    featsT_d = nc.dram_tensor("featsT", [D, N], BF16, kind="ExternalInput")
    rows2_d = nc.dram_tensor("rows2", [D, ROWS_PER_CORE], BF16,
                             kind="ExternalInput")
    sqT_d = nc.dram_tensor("sqT", [D, TJT], F32, kind="ExternalInput")

    strips_d = nc.dram_tensor("strips", [D, RT_PER_CORE * S], F16,
                              kind="ExternalOutput")
    tmins_d = nc.dram_tensor("tmins", [D, N_SHIP * ROWS_PER_CORE], F16,
                             kind="ExternalOutput")
    chains_d = nc.dram_tensor("chains", [D, NB_CHAINS * ROWS_PER_CORE], F16,
                              kind="ExternalOutput")

    modes = _mode_plan(TJT, N_MASK)

    # interleaved emission: two row-tiles up front (ACT has work while the
    # bulk featsT streams in), then one row-tile per ~8 j-tiles
    order = [("row", 0), ("row", 1)]
    jt = 0
    for lt in range(2, RT_PER_CORE):
        take = (TJT * (lt - 1)) // (RT_PER_CORE - 2) - \
               (TJT * (lt - 2)) // (RT_PER_CORE - 2)
        for _ in range(take):
            order.append(("jt", jt))
            jt += 1
        order.append(("row", lt))
    while jt < TJT:
        order.append(("jt", jt))
        jt += 1

    with TileContext(nc) as tc:
        with (
            tc.tile_pool(name="big", bufs=1) as big,
            tc.tile_pool(name="tps", bufs=4, space="PSUM") as tps_pool,
            tc.tile_pool(name="stp", bufs=2) as stp_pool,
            tc.tile_pool(name="shp", bufs=3) as shp_pool,
            tc.tile_pool(name="brn", bufs=2) as brn_pool,
            tc.tile_pool(name="small", bufs=1) as small,
        ):
            featsT = big.tile([D, N], BF16, tag="featsT")
            rows2 = big.tile([D, ROWS_PER_CORE], BF16, tag="rows2")
            sqT = small.tile([D, TJT], F32, tag="sqT")

            # ---- input DMAs: the critical prefix (rows2 + strip cols)
            # loads in parallel on two queues; bulk featsT rides BEHIND the
            # critical items on the same queues so it cannot starve them.
            nc.sync.dma_start(rows2[:, :], rows2_d[:, :])
            nc.gpsimd.dma_start(featsT[:, 0:S], featsT_d[:, 0:S])
            nc.scalar.dma_start(sqT[:, :], sqT_d[:, :])
            # small slice covering the first few j-tiles unblocks the DVE
            # chain path early
            nc.sync.dma_start(featsT[:, S:S + 1280], featsT_d[:, S:S + 1280])
            nc.sync.dma_start(featsT[:, S + 1280:3456],
                              featsT_d[:, S + 1280:3456])
            nc.gpsimd.dma_start(featsT[:, 3456:5824], featsT_d[:, 3456:5824])
            nc.scalar.dma_start(featsT[:, 5824:8192], featsT_d[:, 5824:8192])

            dma_flip = 0
            ship_k = 0
            shipbuf = None
            stripbuf = None
            brun = [[None, None] for _ in range(NB_CHAINS)]
            brun_n = [0] * NB_CHAINS
            bc_counts = [(N_BC + NB_CHAINS - 1 - ci) // NB_CHAINS
                         for ci in range(NB_CHAINS)]
            bc_k = 0
            chainbuf = small.tile([D, NB_CHAINS * ROWS_PER_CORE], F16,
                                  tag="chainbuf")

            def out_dma(dst, src):
                nonlocal dma_flip
                if dma_flip % 2 == 0:
                    nc.gpsimd.dma_start(dst, src)
                else:
                    nc.sync.dma_start(dst, src)
                dma_flip += 1

            for kind, idx in order:
                if kind == "row":
                    lt = idx
                    psA = tps_pool.tile([D, 1024], F32, tag="ps_t",
                                        name=f"psA{lt}")
                    lhsT = rows2[:, 128 * lt:128 * (lt + 1)]
                    nc.tensor.matmul(psA[:, 0:512], lhsT,
                                     featsT[:, 0:512],
                                     start=True, stop=True)
                    nc.tensor.matmul(psA[:, 512:1024], lhsT,
                                     featsT[:, 512:1024],
                                     start=True, stop=True)
                    if lt % 2 == 0:
                        stripbuf = stp_pool.tile([D, 2 * S], F16, tag="st",
                                                 name=f"st{lt}")
                    half = (lt % 2) * S
                    nc.scalar.activation(
                        stripbuf[:, half:half + S], psA[:, :],
                        mybir.ActivationFunctionType.Copy,
                        bias=0.0, scale=1.0)
                    if lt % 2 == 1:
                        out_dma(strips_d[:, (lt - 1) * S:(lt + 1) * S],
                                stripbuf[:, :])
                else:
                    jt = idx
                    j0 = S + 128 * jt
                    ps_t = tps_pool.tile([D, ROWS_PER_CORE], F32, tag="ps_t",
                                         name=f"ps_t{jt}")
                    lhsT = featsT[:, j0:j0 + 128]
                    nc.tensor.matmul(ps_t[:, 0:512], lhsT, rows2[:, 0:512],
                                     start=True, stop=True)
                    nc.tensor.matmul(ps_t[:, 512:1024], lhsT,
                                     rows2[:, 512:1024],
                                     start=True, stop=True)
                    mode = modes[jt]
                    if mode == "BC":
                        # DVE (psum + sq_j) min chain, fp16
                        ci = bc_k % NB_CHAINS
                        bc_k += 1
                        last = brun_n[ci] == bc_counts[ci] - 1
                        if last:
                            run = chainbuf[:, ci * ROWS_PER_CORE:
                                           (ci + 1) * ROWS_PER_CORE]
                        else:
                            run = brn_pool.tile(
                                [D, ROWS_PER_CORE], F16,
                                tag=f"brun{ci}", name=f"brun{jt}")[:, :]
                        if brun_n[ci] == 0:
                            nc.vector.tensor_scalar(
                                out=run, in0=ps_t[:, :],
                                scalar1=sqT[:, jt:jt + 1], scalar2=None,
                                op0=mybir.AluOpType.add)
                        else:
                            nc.vector.scalar_tensor_tensor(
                                out=run, in0=ps_t[:, :],
                                scalar=sqT[:, jt:jt + 1],
                                in1=brun[ci][(brun_n[ci] + 1) % 2],
                                op0=mybir.AluOpType.add,
                                op1=mybir.AluOpType.min)
                        brun[ci][brun_n[ci] % 2] = run
                        brun_n[ci] += 1
                    else:
                        # CS: ACT raw copy into a batched ship buffer
                        # (host adds +sq_j)
                        slot = ship_k % 2
                        if slot == 0:
                            shipbuf = shp_pool.tile(
                                [D, 2 * ROWS_PER_CORE], F16, tag="shipbuf",
                                name=f"shipbuf{ship_k}")
                        nc.scalar.activation(
                            shipbuf[:, slot * ROWS_PER_CORE:
                                    (slot + 1) * ROWS_PER_CORE],
                            ps_t[:, :],
                            mybir.ActivationFunctionType.Copy,
                            bias=0.0, scale=1.0)
                        ship_k += 1
                        if slot == 1:
                            b0 = (ship_k - 2) * ROWS_PER_CORE
                            out_dma(tmins_d[:, b0:b0 + 2 * ROWS_PER_CORE],
                                    shipbuf[:, :])

            # flush a partial ship batch
            rem = ship_k % 2
            if rem:
                b0 = (ship_k - rem) * ROWS_PER_CORE
                out_dma(tmins_d[:, b0:b0 + rem * ROWS_PER_CORE],
                        shipbuf[:, 0:rem * ROWS_PER_CORE])
            # chain finals were written straight into chainbuf
            nc.sync.dma_start(chains_d[:, :], chainbuf[:, :])

    nc.compile()
    return nc


def kernel(feats, labels):
    from concourse.bass_utils import run_bass_kernel_spmd

    feats = np.asarray(feats, dtype=np.float32)
    labels_np = np.asarray(labels).astype(np.int64)

    order = np.argsort(labels_np, kind="stable")
    feats_s = feats[order]
    labels_s = labels_np[order]

    counts = np.bincount(labels_s, minlength=max(int(labels_s.max()) + 1, 1))
    mc = int(counts.max())
    if mc <= 33:
        W = 192
    elif mc <= 65:
        W = 256
    elif mc <= 129:
        W = 384
    elif mc <= 193:
        W = 512
    else:
        raise ValueError(f"class of size {mc} exceeds supported band window")
    WEND = 64 + 128 * 7 + W
    NMASK = max(0, (WEND - S + 127) // 128)

    if W not in _PROGRAM_CACHE:
        _PROGRAM_CACHE[W] = _build_program(W)
    nc = _PROGRAM_CACHE[W]
    TJT = (N - S) // 128

    sq = np.einsum("nd,nd->n", feats_s.astype(np.float64),
                   feats_s.astype(np.float64)).astype(np.float32)

    in_maps = []
    locs = []
    for c in range(NCORES):
        rot = (ROWS_PER_CORE * c - W // 2) % N
        loc = (rot + np.arange(N)) % N          # local col -> global sorted row
        locs.append(loc)
        featsT_c = np.ascontiguousarray(
            feats_s[loc].T.astype(ml_dtypes.bfloat16))
        rows2_c = np.ascontiguousarray(
            (-2.0 * feats_s[ROWS_PER_CORE * c:ROWS_PER_CORE * (c + 1)])
            .T.astype(ml_dtypes.bfloat16))
        sq_loc = sq[loc]
        sqT_c = np.ascontiguousarray(
            sq_loc[S:S + TJT * 128].reshape(TJT, 128).T.astype(np.float32))
        in_maps.append({
            "featsT": featsT_c,
            "rows2": rows2_c,
            "sqT": sqT_c,
        })

    res = run_bass_kernel_spmd(nc, in_maps, core_ids=list(range(NCORES)))

    modes = _mode_plan(TJT, NMASK)
    cs_jts = np.array([j for j in range(TJT) if modes[j] == "CS"])

    neg_raw = np.empty(N, dtype=np.float32)
    pos_raw = np.empty(N, dtype=np.float32)
    for c in range(NCORES):
        base = ROWS_PER_CORE * c
        loc = locs[c]
        sq_loc = sq[loc]
        rows_lab = labels_s[base:base + ROWS_PER_CORE]
        # shipped tiles are RAW -2 x_j.x_i: add +sq_j here; chain tiles
        # already carry it from the device stt
        tm = res.results[c]["tmins"]             # [128, nship*1024] fp16
        ch = res.results[c]["chains"]            # [128, 3*1024] fp16
        nship = tm.shape[1] // ROWS_PER_CORE
        nchain = ch.shape[1] // ROWS_PER_CORE
        sq_ship = sq_loc[S + cs_jts[None, :] * 128 +
                         np.arange(128)[:, None]]          # [128, nship]
        tmf = tm.reshape(D, nship, ROWS_PER_CORE).astype(np.float32)
        tmf += sq_ship[:, :, None]
        # band ship-tiles: mask same-class entries (and collect their
        # hardest-pos candidates)
        pos_extra = np.full(ROWS_PER_CORE, -np.inf, dtype=np.float32)
        for k in range(NMASK):
            col_lab = labels_s[loc[S + 128 * k:S + 128 * (k + 1)]]
            msame = col_lab[:, None] == rows_lab[None, :]   # [128, 1024]
            tile = tmf[:, k, :]
            pos_extra = np.maximum(
                pos_extra, np.where(msame, tile, -np.inf).max(axis=0))
            tmf[:, k, :] = np.where(msame, np.inf, tile)
        tmin = np.minimum(tmf.min(axis=(0, 1)),
                          ch.reshape(D, nchain, ROWS_PER_CORE)
                            .astype(np.float32).min(axis=(0, 1)))
        # strip: raw -2 x_i . x_j for local cols [0, S)
        stp = res.results[c]["strips"].reshape(D, RT_PER_CORE, S)
        raw = stp.transpose(1, 0, 2).reshape(ROWS_PER_CORE, S) \
                 .astype(np.float32)
        vals = raw + sq_loc[None, 0:S]
        same = rows_lab[:, None] == labels_s[loc[0:S]][None, :]
        negs = np.where(same, np.float32(np.inf), vals).min(axis=1)
        poss = np.where(same, vals, np.float32(-np.inf)).max(axis=1)
        neg_raw[base:base + ROWS_PER_CORE] = np.minimum(negs, tmin)
        pos_raw[base:base + ROWS_PER_CORE] = np.maximum(poss, pos_extra)

    hn_sq = np.maximum(neg_raw + sq, 0.0).astype(np.float32)
    hp_sq = np.maximum(pos_raw + sq, 0.0).astype(np.float32)
    eps = np.float32(1e-12)
    hn = np.where(hn_sq > eps, np.sqrt(hn_sq), np.float32(0.0))
    hp = np.where(hp_sq > eps, np.sqrt(hp_sq), np.float32(0.0))

    cnt_per_row = counts[labels_s]
    valid = (cnt_per_row >= 2) & (cnt_per_row < N)
    diff = np.where(valid, hp - hn, np.float32(0.0))
    per_row = np.maximum(diff + np.float32(MARGIN), np.float32(0.0))
    per_row = np.where(valid, per_row, np.float32(0.0)).astype(np.float32)
    cnt = np.float32(valid.sum())
    if cnt > 0:
        loss = np.float32(per_row.sum(dtype=np.float32) /
                          max(cnt, np.float32(1.0)))
    else:
        loss = np.float32(0.0)
    return np.float32(loss)
